# revision 13
# baseline (speedup 1.0000x reference)
"""Trainium2 Bass kernel for nn_CubicalModel_ISM_norm.

Reference pipeline:
  Ip = reshape(tensordot(I, p, 1), [28, 28])          # device math
  inds = cubical_persistence(Ip)                       # host pure_callback
  dgm = Ip[inds[:, 0], inds[:, 1]].reshape(-1, 2)      # device gather

Device program (replicated SPMD on 8 cores; batch=1 so the data-parallel
sharding hint degenerates to replication — core 0's output is returned).

The projection+gather is one bilinear form over the raw image:
  out[n] = sum_{p,c} I[p, c] * W[p, c, n],
  W[p, c, n] = [p == flat[n]//7] * (p0*[c == 2*(flat[n]%7)] +
                                    p1*[c == 2*(flat[n]%7)+1])
which is SEPARABLE: W[p,c,n] = R[p,n] * S[c,n]. So the device runs:
  1. three parallel input DMAs (I [112,14] on sync, R [112,100] on scalar,
     S [14,100] on gpsimd) + a gpsimd memset of a ones column
  2. PE mm1: psum1[14,100] = I.T @ R     (row-gather: psum1[c,n] = I[r_n,c])
  3. DVE:    mask[14,100]  = psum1 * S   (column select × projection p)
  4. PE mm2: psum2[1,100]  = ones.T @ mask  (fold the 14 rows)
  5. DVE copies psum2 -> SBUF; one contiguous 400 B DMA out

Design notes (all measured on HW):
  - PE fp32 is dual-pass (LOW/HIGH) and passes one-hot-selected values
    through bit-exactly; the whole chain is bit-identical to numpy-f32
    I[r,2c]*p0 + I[r,2c+1]*p1.
  - The factored masks are 50 KB total vs 627 KB for the unfactored W
    (which cost ~4 us of DMA) and replace 14 chunked accumulating
    matmuls (~4.8 us of dual-pass PE time) with two small matmuls.
  - The result never leaves partition 0..13, so no 100-partition-strided
    DMA (~50 ns per 4 B element) appears anywhere; the output store is a
    single 400 B burst.
  - No dependent back-to-back DVE ops (the DVE pipeline has a same-engine
    RAW hazard on short ops that raw Bass does not guard); every producer/
    consumer pair here is cross-engine and semaphore-separated.
  - The program is input-independent, so the compiled NEFF caches across
    calls.

The persistence indices are computed on host (numpy) from the same f32
contraction; they only depend on the *ordering* of Ip values (min sorted
gap 5.0e-6 for this input distribution vs ~2.4e-7 cross-backend float
drift), so host/device float differences cannot flip them.
"""

import numpy as np

H, W = 28, 28
DIM = 1
CARD = 50
D = 2
N_CORES = 8

NPIX = H * W            # 784
NG = 2 * CARD           # 100 gathered values
PPART = 112             # partition rows of I; 7 pixels (14 floats) each
CHUNK = 14              # floats per partition row


# ---------------------------------------------------------------------------
# Host-side cubical persistence (mirrors the reference's pure_callback)
# ---------------------------------------------------------------------------
def _cubical(X, dim=DIM, card=CARD):
    X = np.asarray(X, dtype=np.float64)
    Hh, Ww = X.shape
    A, Bc = 2 * Hh - 1, 2 * Ww - 1
    n = A * Bc
    F = np.empty(n)
    P = np.empty(n, np.int64)
    dms = np.empty(n, np.int64)

    def cands(a, lim):
        if a % 2 == 1:
            return [(a - 1) // 2]
        return [r for r in (a // 2 - 1, a // 2) if 0 <= r < lim]

    for a in range(A):
        ris = cands(a, Hh)
        for b in range(Bc):
            cjs = cands(b, Ww)
            best = None
            bp = 0
            for r in ris:
                for c in cjs:
                    v = X[r, c]
                    if best is None or v < best:
                        best = v
                        bp = r * Ww + c
            idx = a * Bc + b
            F[idx] = best
            P[idx] = bp
            dms[idx] = (a & 1) + (b & 1)

    order = np.lexsort((np.arange(n), dms, F))
    pos = np.empty(n, np.int64)
    pos[order] = np.arange(n)

    def faces(cell):
        a, b = divmod(cell, Bc)
        fs = []
        if a & 1:
            fs += [(a - 1) * Bc + b, (a + 1) * Bc + b]
        if b & 1:
            fs += [a * Bc + b - 1, a * Bc + b + 1]
        return fs

    Rcols = {}
    low_to_col = {}
    pairs = []
    for j in range(n):
        cell = order[j]
        col = set(int(pos[f]) for f in faces(cell))
        while col:
            l = max(col)
            k = low_to_col.get(l)
            if k is None:
                break
            col ^= Rcols[k]
        if col:
            l = max(col)
            low_to_col[l] = j
            Rcols[j] = col
            pairs.append((l, j))

    rows, pers = [], []
    for (i, j) in pairs:
        bc, dc = order[i], order[j]
        if dms[bc] != dim:
            continue
        pr = F[dc] - F[bc]
        if pr <= 0:
            continue
        rows.append((int(P[bc]), int(P[dc])))
        pers.append(pr)
    if rows:
        srt = np.argsort(np.asarray(pers))[::-1]
        rows = [rows[k] for k in srt]

    inds = []
    for k in range(min(card, len(rows))):
        bi, di = rows[k]
        inds += [bi // Ww, bi % Ww, di // Ww, di % Ww]
    inds += [0] * (2 * D * card - len(inds))
    return np.asarray(inds, dtype=np.int32)


# ---------------------------------------------------------------------------
# Bass program (input-independent; built once per process)
# ---------------------------------------------------------------------------
_PROGRAM_CACHE = {}


def _build_program():
    if "nc" in _PROGRAM_CACHE:
        return _PROGRAM_CACHE["nc"]

    from concourse import bass, mybir

    nc = bass.Bass()

    # I and R share 112 partitions -> packed into one [112, 14+100] input so
    # a single DMA on the sync HWDGE queue loads both. S rides the scalar
    # HWDGE queue (gpsimd's SWDGE is busy with framework memsets until well
    # into the body and issued ~0.7 us late when used for inputs).
    ir_dram = nc.declare_dram_parameter("IR", [PPART, CHUNK + NG],
                                        mybir.dt.float32, isOutput=False)
    s_dram = nc.declare_dram_parameter("S", [CHUNK, NG],
                                       mybir.dt.float32, isOutput=False)
    out_dram = nc.declare_dram_parameter("out", [1, NG], mybir.dt.float32,
                                         isOutput=True)

    with (
        nc.sbuf_tensor("ir_s", [PPART, CHUNK + NG], mybir.dt.float32) as ir_s,
        nc.sbuf_tensor("s_s", [CHUNK, NG], mybir.dt.float32) as s_s,
        nc.sbuf_tensor("ones_s", [CHUNK, 1], mybir.dt.float32) as ones_s,
        nc.sbuf_tensor("mask_s", [CHUNK, NG], mybir.dt.float32) as mask_s,
        nc.sbuf_tensor("out_s", [1, NG], mybir.dt.float32) as out_s,
        nc.psum_tensor("acc1", [CHUNK, NG], mybir.dt.float32) as acc1,
        nc.psum_tensor("acc2", [1, NG], mybir.dt.float32) as acc2,
        nc.semaphore("bsem") as bsem,     # I/R load
        nc.semaphore("ssem") as ssem,     # S load + output store
        nc.semaphore("csem") as csem,     # compute chain
        nc.Block() as block,
    ):
        @block.sync
        def _(sync):
            sync.dma_start(out=ir_s[0:56, :], in_=ir_dram[0:56, :]).then_inc(bsem, 16)
            sync.dma_start(out=s_s[:, :], in_=s_dram[:, :]).then_inc(ssem, 16)
            sync.wait_ge(csem, 5)
            sync.dma_start(out=out_dram[:, :], in_=out_s[:, :]).then_inc(ssem, 16)
            sync.wait_ge(ssem, 32)

        @block.scalar
        def _(scalar):
            scalar.dma_start(out=ir_s[56:PPART, :],
                             in_=ir_dram[56:PPART, :]).then_inc(bsem, 16)

        @block.tensor
        def _(tensor):
            tensor.wait_ge(bsem, 32)      # both I/R halves (S not needed by PE)
            tensor.wait_ge(csem, 1)       # ones ready
            tensor.matmul(out=acc1[:, :], lhsT=ir_s[:, 0:CHUNK],
                          rhs=ir_s[:, CHUNK:CHUNK + NG],
                          start=True, stop=True).then_inc(csem, 1)   # -> 2
            tensor.wait_ge(csem, 3)       # mask written
            tensor.matmul(out=acc2[0:1, :], lhsT=ones_s[:, 0:1],
                          rhs=mask_s[:, :],
                          start=True, stop=True).then_inc(csem, 1)   # -> 4

        @block.vector
        def _(vector):
            vector.memset(ones_s[:, :], 1.0).then_inc(csem, 1)       # -> 1
            vector.wait_ge(csem, 2)       # mm1 done
            vector.wait_ge(ssem, 16)      # S loaded
            vector.tensor_tensor(out=mask_s[:, :], in0=acc1[:, :],
                                 in1=s_s[:, :],
                                 op=mybir.AluOpType.mult).then_inc(csem, 1)  # -> 3
            vector.wait_ge(csem, 4)       # mm2 done
            vector.tensor_copy(out_s[0:1, :], acc2[0:1, :]).then_inc(csem, 1)  # -> 5

    _PROGRAM_CACHE["nc"] = nc
    return nc


def _host_prep(I, p):
    """Persistence indices + factored gather/projection masks (host side of
    the reference's pure_callback)."""
    ip_host = (I.reshape(-1, 2) @ p).reshape(H, W)
    inds = _cubical(ip_host).reshape(-1, D)            # [100, 2] (row, col)
    flat = inds[:, 0] * W + inds[:, 1]                 # [100] flat pixel idx

    n = np.arange(NG)
    R = np.zeros((PPART, NG), dtype=np.float32)
    R[flat // 7, n] = 1.0
    S = np.zeros((CHUNK, NG), dtype=np.float32)
    S[2 * (flat % 7), n] = p[0, 0]
    S[2 * (flat % 7) + 1, n] = p[1, 0]
    IR = np.concatenate([I.reshape(PPART, CHUNK), R], axis=1)
    return IR, S


# ---------------------------------------------------------------------------
# Entry point
# ---------------------------------------------------------------------------
def kernel(I, p):
    from concourse.bass_utils import run_bass_kernel_spmd

    I = np.ascontiguousarray(np.asarray(I, dtype=np.float32))
    p = np.ascontiguousarray(np.asarray(p, dtype=np.float32))

    IR, S = _host_prep(I, p)
    nc = _build_program()

    in_map = {"IR": IR, "S": S}
    core_ids = list(range(N_CORES))
    res = run_bass_kernel_spmd(nc, [dict(in_map) for _ in core_ids], core_ids)
    out = np.asarray(res.results[0]["out"], dtype=np.float32).reshape(-1, 2)
    return out


# revision 15
# speedup vs baseline: 1.0225x; 1.0225x over previous
"""Trainium2 Bass kernel for nn_CubicalModel_ISM_norm.

Reference pipeline:
  Ip = reshape(tensordot(I, p, 1), [28, 28])          # device math
  inds = cubical_persistence(Ip)                       # host pure_callback
  dgm = Ip[inds[:, 0], inds[:, 1]].reshape(-1, 2)      # device gather

Device program (replicated SPMD on 8 cores; batch=1 so the data-parallel
sharding hint degenerates to replication — core 0's output is returned).

The projection+gather is one bilinear form over the raw image:
  out[n] = sum_{p,c} I[p, c] * W[p, c, n],
  W[p, c, n] = [p == flat[n]//7] * (p0*[c == 2*(flat[n]%7)] +
                                    p1*[c == 2*(flat[n]%7)+1])
which is SEPARABLE: W[p,c,n] = R[p,n] * S[c,n]. So the device runs:
  1. three parallel input DMAs (I [112,14] on sync, R [112,100] on scalar,
     S [14,100] on gpsimd) + a gpsimd memset of a ones column
  2. PE mm1: psum1[14,100] = I.T @ R     (row-gather: psum1[c,n] = I[r_n,c])
  3. DVE:    mask[14,100]  = psum1 * S   (column select × projection p)
  4. PE mm2: psum2[1,100]  = ones.T @ mask  (fold the 14 rows)
  5. DVE copies psum2 -> SBUF; one contiguous 400 B DMA out

Design notes (all measured on HW):
  - PE fp32 is dual-pass (LOW/HIGH) and passes one-hot-selected values
    through bit-exactly; the whole chain is bit-identical to numpy-f32
    I[r,2c]*p0 + I[r,2c+1]*p1.
  - The factored masks are 50 KB total vs 627 KB for the unfactored W
    (which cost ~4 us of DMA) and replace 14 chunked accumulating
    matmuls (~4.8 us of dual-pass PE time) with two small matmuls.
  - The result never leaves partition 0..13, so no 100-partition-strided
    DMA (~50 ns per 4 B element) appears anywhere; the output store is a
    single 400 B burst.
  - No dependent back-to-back DVE ops (the DVE pipeline has a same-engine
    RAW hazard on short ops that raw Bass does not guard); every producer/
    consumer pair here is cross-engine and semaphore-separated.
  - The program is input-independent, so the compiled NEFF caches across
    calls.

The persistence indices are computed on host (numpy) from the same f32
contraction; they only depend on the *ordering* of Ip values (min sorted
gap 5.0e-6 for this input distribution vs ~2.4e-7 cross-backend float
drift), so host/device float differences cannot flip them.
"""

import numpy as np

H, W = 28, 28
DIM = 1
CARD = 50
D = 2
N_CORES = 8

NPIX = H * W            # 784
NG = 2 * CARD           # 100 gathered values
PPART = 112             # partition rows of I; 7 pixels (14 floats) each
CHUNK = 14              # floats per partition row


# ---------------------------------------------------------------------------
# Host-side cubical persistence (mirrors the reference's pure_callback)
# ---------------------------------------------------------------------------
def _cubical(X, dim=DIM, card=CARD):
    X = np.asarray(X, dtype=np.float64)
    Hh, Ww = X.shape
    A, Bc = 2 * Hh - 1, 2 * Ww - 1
    n = A * Bc
    F = np.empty(n)
    P = np.empty(n, np.int64)
    dms = np.empty(n, np.int64)

    def cands(a, lim):
        if a % 2 == 1:
            return [(a - 1) // 2]
        return [r for r in (a // 2 - 1, a // 2) if 0 <= r < lim]

    for a in range(A):
        ris = cands(a, Hh)
        for b in range(Bc):
            cjs = cands(b, Ww)
            best = None
            bp = 0
            for r in ris:
                for c in cjs:
                    v = X[r, c]
                    if best is None or v < best:
                        best = v
                        bp = r * Ww + c
            idx = a * Bc + b
            F[idx] = best
            P[idx] = bp
            dms[idx] = (a & 1) + (b & 1)

    order = np.lexsort((np.arange(n), dms, F))
    pos = np.empty(n, np.int64)
    pos[order] = np.arange(n)

    def faces(cell):
        a, b = divmod(cell, Bc)
        fs = []
        if a & 1:
            fs += [(a - 1) * Bc + b, (a + 1) * Bc + b]
        if b & 1:
            fs += [a * Bc + b - 1, a * Bc + b + 1]
        return fs

    Rcols = {}
    low_to_col = {}
    pairs = []
    for j in range(n):
        cell = order[j]
        col = set(int(pos[f]) for f in faces(cell))
        while col:
            l = max(col)
            k = low_to_col.get(l)
            if k is None:
                break
            col ^= Rcols[k]
        if col:
            l = max(col)
            low_to_col[l] = j
            Rcols[j] = col
            pairs.append((l, j))

    rows, pers = [], []
    for (i, j) in pairs:
        bc, dc = order[i], order[j]
        if dms[bc] != dim:
            continue
        pr = F[dc] - F[bc]
        if pr <= 0:
            continue
        rows.append((int(P[bc]), int(P[dc])))
        pers.append(pr)
    if rows:
        srt = np.argsort(np.asarray(pers))[::-1]
        rows = [rows[k] for k in srt]

    inds = []
    for k in range(min(card, len(rows))):
        bi, di = rows[k]
        inds += [bi // Ww, bi % Ww, di // Ww, di % Ww]
    inds += [0] * (2 * D * card - len(inds))
    return np.asarray(inds, dtype=np.int32)


# ---------------------------------------------------------------------------
# Bass program (input-independent; built once per process)
# ---------------------------------------------------------------------------
_PROGRAM_CACHE = {}


def _build_program():
    if "nc" in _PROGRAM_CACHE:
        return _PROGRAM_CACHE["nc"]

    from concourse import bass, mybir

    nc = bass.Bass()

    # I and R share 112 partitions -> packed into one [112, 14+100] input so
    # a single DMA on the sync HWDGE queue loads both. S rides the scalar
    # HWDGE queue (gpsimd's SWDGE is busy with framework memsets until well
    # into the body and issued ~0.7 us late when used for inputs).
    ir_dram = nc.declare_dram_parameter("IR", [PPART, CHUNK + NG],
                                        mybir.dt.float32, isOutput=False)
    s_dram = nc.declare_dram_parameter("S", [CHUNK, NG],
                                       mybir.dt.float32, isOutput=False)
    out_dram = nc.declare_dram_parameter("out", [1, NG], mybir.dt.float32,
                                         isOutput=True)

    with (
        nc.sbuf_tensor("ir_s", [PPART, CHUNK + NG], mybir.dt.float32) as ir_s,
        nc.sbuf_tensor("s_s", [CHUNK, NG], mybir.dt.float32) as s_s,
        nc.sbuf_tensor("ones_s", [CHUNK, 1], mybir.dt.float32) as ones_s,
        nc.sbuf_tensor("mask_s", [CHUNK, NG], mybir.dt.float32) as mask_s,
        nc.sbuf_tensor("out_s", [1, NG], mybir.dt.float32) as out_s,
        nc.psum_tensor("acc1", [CHUNK, NG], mybir.dt.float32) as acc1,
        nc.psum_tensor("acc2", [1, NG], mybir.dt.float32) as acc2,
        nc.semaphore("bsem") as bsem,     # I/R load
        nc.semaphore("ssem") as ssem,     # S load + output store
        nc.semaphore("csem") as csem,     # compute chain
        nc.Block() as block,
    ):
        @block.sync
        def _(sync):
            sync.dma_start(out=ir_s[:, :], in_=ir_dram[:, :]).then_inc(bsem, 16)
            sync.wait_ge(csem, 5)
            sync.dma_start(out=out_dram[:, :], in_=out_s[:, :]).then_inc(ssem, 16)
            sync.wait_ge(ssem, 32)

        @block.scalar
        def _(scalar):
            scalar.dma_start(out=s_s[:, :], in_=s_dram[:, :]).then_inc(ssem, 16)

        @block.tensor
        def _(tensor):
            tensor.wait_ge(bsem, 16)      # I/R loaded (S not needed by PE)
            tensor.wait_ge(csem, 1)       # ones ready
            tensor.matmul(out=acc1[:, :], lhsT=ir_s[:, 0:CHUNK],
                          rhs=ir_s[:, CHUNK:CHUNK + NG],
                          start=True, stop=True).then_inc(csem, 1)   # -> 2
            tensor.wait_ge(csem, 3)       # mask written
            tensor.matmul(out=acc2[0:1, :], lhsT=ones_s[:, 0:1],
                          rhs=mask_s[:, :],
                          start=True, stop=True).then_inc(csem, 1)   # -> 4

        @block.vector
        def _(vector):
            vector.memset(ones_s[:, :], 1.0).then_inc(csem, 1)       # -> 1
            vector.wait_ge(csem, 2)       # mm1 done
            vector.wait_ge(ssem, 16)      # S loaded
            vector.tensor_tensor(out=mask_s[:, :], in0=acc1[:, :],
                                 in1=s_s[:, :],
                                 op=mybir.AluOpType.mult).then_inc(csem, 1)  # -> 3
            vector.wait_ge(csem, 4)       # mm2 done
            vector.tensor_copy(out_s[0:1, :], acc2[0:1, :]).then_inc(csem, 1)  # -> 5

    _PROGRAM_CACHE["nc"] = nc
    return nc


def _host_prep(I, p):
    """Persistence indices + factored gather/projection masks (host side of
    the reference's pure_callback)."""
    ip_host = (I.reshape(-1, 2) @ p).reshape(H, W)
    inds = _cubical(ip_host).reshape(-1, D)            # [100, 2] (row, col)
    flat = inds[:, 0] * W + inds[:, 1]                 # [100] flat pixel idx

    n = np.arange(NG)
    R = np.zeros((PPART, NG), dtype=np.float32)
    R[flat // 7, n] = 1.0
    S = np.zeros((CHUNK, NG), dtype=np.float32)
    S[2 * (flat % 7), n] = p[0, 0]
    S[2 * (flat % 7) + 1, n] = p[1, 0]
    IR = np.concatenate([I.reshape(PPART, CHUNK), R], axis=1)
    return IR, S


# ---------------------------------------------------------------------------
# Entry point
# ---------------------------------------------------------------------------
def kernel(I, p):
    from concourse.bass_utils import run_bass_kernel_spmd

    I = np.ascontiguousarray(np.asarray(I, dtype=np.float32))
    p = np.ascontiguousarray(np.asarray(p, dtype=np.float32))

    IR, S = _host_prep(I, p)
    nc = _build_program()

    in_map = {"IR": IR, "S": S}
    core_ids = list(range(N_CORES))
    res = run_bass_kernel_spmd(nc, [dict(in_map) for _ in core_ids], core_ids)
    out = np.asarray(res.results[0]["out"], dtype=np.float32).reshape(-1, 2)
    return out
